# revision 16
# baseline (speedup 1.0000x reference)
"""Trainium2 Bass kernel for AdaptConv-style GNN message passing.

Reference computation (per batch element b):
    h   = x @ W.T + b                       # [N, OUT]
    hn  = h / max(||h||_row, 1e-12)         # row-wise L2 normalize
    cos = hn @ hn.T                         # [N, N]
    out = relu((edge_weight * cos) @ h)     # [N, OUT]

Sharding: pure data-parallel over batch B=8 across the 8 NeuronCores
(no collectives).

Host-side preprocessing (linear+normalize is 0.8% of FLOPs, folded into
the input layout pass):
    hh = [hn.T | (h/255)-rowmajor-per-band]  bf16 [128, 2*N]
    edge weights are stored pre-transposed and pre-scaled by 255, split
    by gate class:
      etq uint8 [128, ...]: round(255*ew) blocks for DVE-direct and
          GpSimd gate bands (8-bit fixed point, sigma ~0.2%)
      eth bf16  [128, ...]: 255*ew blocks for the ScalarE-copy+DVE bands
          (bf16 keeps the DVE mul in 2x packed mode)
    The 1/255 de-scale rides for free in the hp half of hh.

On-chip dataflow per core: 4 column passes x 16 q-bands, FD=512
matmuls (bf16, fp32 PSUM):
    cos[q', 512p] = hnT[:,q]^T @ hnT[:, pass-cols]   (1 MM -> 1-bank PSUM)
    gt = et[q,p] * cos    per-band gate rotated over DVE-direct /
                          ScalarE-copy+DVE(deferred) / ScalarE-copy+GpSimd
    outT_p[:, :] += hp[q]^T @ gt                      (1 MM, PSUM accum)
    relu epilogue per pass in 2 halves (ScalarE, bf16) + DMA out.
outT is double-buffered (2x1 PSUM bank) so pass p+1 aggs never wait on
pass p's relu; cos tiles rotate over 6 single-bank PSUM buffers.

Warmup: ~32 scratch FD=128 matmuls open the HAM clock window (PE at
2.4GHz when real MMs start) and cover the hnT DMA flight time.

DMA: all on the Sync HWDGE ring, issued in first-use order.
"""

import ml_dtypes
import numpy as np

import concourse.mybir as mybir
import concourse.tile as tile
from concourse import bacc
from concourse.bass_utils import run_bass_kernel_spmd

B, N, IN, OUT = 8, 2048, 128, 128
NQ = N // 128          # 16 q-bands
NP = N // 512          # 4 column passes
FP32 = mybir.dt.float32
BF16 = mybir.dt.bfloat16
U8 = mybir.dt.uint8
AF = mybir.ActivationFunctionType
EPS = 1e-12

CORE_IDS = list(range(8))

N_WARMUP = 30  # ~3.2us of cold FD=128 matmuls: covers the HAM window and
               # keeps the PE busy until the hnT DMA lands
LAG = 8        # agg matmuls trail gates by this many bands (absorbs DVE/
               # ScalarE gate-queue jitter at ~95% engine occupancy)

# Gate engine class per (pass, band): v = DVE fp32-direct (u8 et),
# sv = ScalarE copy + deferred DVE bf16 mul (bf16 et), sg = ScalarE copy +
# GpSimd mul (u8 et).  Tails are v/sv so relu never waits on GpSimd; sg
# spaced >=2 so consecutive GpSimd muls don't queue.  Pass 0 pushes all
# sv bands late so the bf16 eth DMA can ride behind the u8/hh chunks
# during the slow DMA ramp.  Last pass drops a late sg (GpSimd latency
# unhideable at the kernel tail).
PATS = [
    ["v", "v", "sg", "v", "sg", "v", "sg", "v",
     "sg", "sv", "sv", "sg", "sv", "sv", "v", "v"],
    ["v", "v", "sg", "sv", "v", "sg", "sv", "v",
     "sg", "sv", "v", "sg", "sv", "sg", "v", "v"],
    ["v", "v", "sg", "sv", "v", "sg", "sv", "v",
     "sg", "sv", "v", "sg", "sv", "sg", "v", "v"],
    ["v", "v", "sg", "sv", "v", "sg", "sv", "v",
     "sg", "sv", "v", "sg", "sv", "sv", "v", "v"],
]

# per-pass column offsets of each band's et block inside etq (u8) / eth
# (bf16), in band order; blocks are stored in consumption order.
ETQ_OFF = []  # [pass][band] -> col offset or None
ETH_OFF = []
ETQ_PASS = []  # cols per pass in etq
ETH_PASS = []
for _p in range(NP):
    qo, ho = [], []
    qc = hc = 0
    for _q in range(NQ):
        if PATS[_p][_q] == "sv":
            ho.append(hc)
            qo.append(None)
            hc += 512
        else:
            qo.append(qc)
            ho.append(None)
            qc += 512
    ETQ_OFF.append(qo)
    ETH_OFF.append(ho)
    ETQ_PASS.append(qc)
    ETH_PASS.append(hc)
ETQ_BASE = [sum(ETQ_PASS[:p]) for p in range(NP + 1)]
ETH_BASE = [sum(ETH_PASS[:p]) for p in range(NP + 1)]


def build_nc():
    from contextlib import ExitStack

    nc = bacc.Bacc("TRN2", target_bir_lowering=False, debug=False, num_devices=8)

    hh = nc.dram_tensor("hh", [128, 2 * N], BF16, kind="ExternalInput").ap()
    etq = nc.dram_tensor("etq", [128, ETQ_BASE[NP]], U8, kind="ExternalInput").ap()
    eth = nc.dram_tensor("eth", [128, ETH_BASE[NP]], BF16, kind="ExternalInput").ap()
    out = nc.dram_tensor("out", [OUT, N], BF16, kind="ExternalOutput").ap()

    with tile.TileContext(nc) as tc, ExitStack() as ctx:
        singles = ctx.enter_context(tc.tile_pool(name="singles", bufs=1))
        gtp = ctx.enter_context(tc.tile_pool(name="gtp", bufs=10))
        csp = ctx.enter_context(tc.tile_pool(name="csp", bufs=6))
        cps_pool = ctx.enter_context(tc.tile_pool(name="cps", bufs=6, space="PSUM"))
        outp = ctx.enter_context(tc.tile_pool(name="outp", bufs=2, space="PSUM"))

        hh_sb = singles.tile([128, 2 * N], BF16, tag="hh_sb")
        etq_sb = singles.tile([128, ETQ_BASE[NP]], U8, tag="etq_sb")
        eth_sb = singles.tile([128, ETH_BASE[NP]], BF16, tag="eth_sb")
        out_sb = singles.tile([OUT, N], BF16, tag="out_sb")
        wsc = singles.tile([128, 128], BF16, tag="wsc")

        # warmup scratch memset on DVE (ready earliest) so warmup matmuls
        # start right after engine init; the tiny gpsimd tensor_mul forces
        # GpSimd's tensor-op library load now instead of in front of the
        # first real gate (kept off wsc so the PE does not wait on GpSimd).
        nc.vector.memset(wsc[:], 0.0)
        lib = singles.tile([1, 2], BF16, tag="lib")
        nc.vector.memset(lib[:], 0.0)
        nc.gpsimd.tensor_mul(lib[0:1, 0:2], lib[0:1, 0:2], lib[0:1, 0:2])

        # ---- DMAs on BOTH HWDGE rings.  The early ramp is per-transfer
        # latency-bound (~1us/transfer regardless of size), so two rings
        # nearly double early delivery — but BOTH ring heads must carry
        # urgent data (a bulk-loaded second ring steals early bandwidth).
        # Sync ring: hh (matmul operands) then pass 1-3 u8 blocks.
        # Scalar ring: pass-0 edge blocks, then pass 2-3 bf16 blocks. ----
        def etq_dma(p, c0, c1, eng=None):
            csl = slice(ETQ_BASE[p] + c0, ETQ_BASE[p] + c1)
            (eng or nc.sync).dma_start(etq_sb[:, csl], etq[:, csl])

        def eth_dma(p, eng=None):
            csl = slice(ETH_BASE[p], ETH_BASE[p + 1])
            (eng or nc.sync).dma_start(eth_sb[:, csl], eth[:, csl])

        def hh_dma(c0, c1):
            nc.sync.dma_start(hh_sb[:, c0:c1], hh[:, c0:c1])

        # scalar ring (issued first so they sit at the ACT queue head,
        # ahead of the gate copies which are not needed until ~13us)
        etq_dma(0, 0, 512 * 3, nc.scalar)         # pass-0 u8, bands 0-2
        etq_dma(0, 512 * 3, 512 * 6, nc.scalar)   # pass-0 u8, bands 3-5
        etq_dma(0, 512 * 6, 512 * 9, nc.scalar)   # pass-0 u8, bands 6-8
        eth_dma(0, nc.scalar)                     # pass-0 bf16 (sv) blocks
        etq_dma(0, 512 * 9, ETQ_PASS[0], nc.scalar)  # pass-0 u8, b11,14,15
        # sync ring
        hh_dma(0, 512)                # pass-0 rhs + lhsT bands 0-3
        hh_dma(512, 1024)             # lhsT bands 4-7 + pass-1 rhs
        hh_dma(2048, 2560)            # hp bands 0-3 (aggs start LAG bands in)
        hh_dma(1024, 2048)            # lhsT bands 8-15 + pass 2/3 rhs
        hh_dma(2560, 3072)            # hp bands 4-7
        hh_dma(3072, 4096)            # hp bands 8-15
        etq_dma(1, 0, 512 * 9)        # pass-1 u8 blocks 0-8 (bands 0-11)
        eth_dma(1)
        etq_dma(1, 512 * 9, ETQ_PASS[1])  # pass-1 u8 blocks, bands 13-15
        etq_dma(2, 0, ETQ_PASS[2])
        eth_dma(2, nc.scalar)
        etq_dma(3, 0, ETQ_PASS[3])
        eth_dma(3, nc.scalar)

        # ---- HAM warmup ----
        outT0 = outp.tile([OUT, 512], FP32, tag="outT", name="outT0")
        for _ in range(N_WARMUP):
            nc.tensor.matmul(
                outT0[:, 0:128], wsc[:], wsc[:],
                start=True, stop=True, skip_group_check=True,
            )

        # ---- main loop ----
        for p in range(NP):
            outT = outT0 if p == 0 else outp.tile(
                [OUT, 512], FP32, tag="outT", name=f"outT{p}"
            )
            PAT = PATS[p]
            rhs = hh_sb[:, p * 512 : (p + 1) * 512]
            pend = []
            defer = []

            def emit_agg(q, gt, stop):
                hpq = hh_sb[:, N + q * 128 : N + (q + 1) * 128]
                nc.tensor.matmul(
                    outT[:], hpq, gt[:],
                    start=(q == 0), stop=stop, skip_group_check=True,
                )

            for q in range(NQ):
                hnq = hh_sb[:, q * 128 : (q + 1) * 128]
                cos = cps_pool.tile([128, 512], FP32, tag="cos", name=f"cos{p}_{q}")
                nc.tensor.matmul(cos[:], hnq, rhs, start=True, stop=True)
                for dgt, dcsb, dets in defer:
                    nc.vector.tensor_mul(dgt[:], dcsb[:], dets)
                defer = []
                gt = gtp.tile([128, 512], BF16, tag="gt", name=f"gt{p}_{q}")
                cls = PAT[q]
                if cls == "v":
                    ets = etq_sb[
                        :, ETQ_BASE[p] + ETQ_OFF[p][q] : ETQ_BASE[p] + ETQ_OFF[p][q] + 512
                    ]
                    nc.vector.tensor_mul(gt[:], cos[:], ets)
                elif cls == "sg":
                    ets = etq_sb[
                        :, ETQ_BASE[p] + ETQ_OFF[p][q] : ETQ_BASE[p] + ETQ_OFF[p][q] + 512
                    ]
                    csb = csp.tile([128, 512], BF16, tag="csb", name=f"csb{p}_{q}")
                    nc.scalar.copy(csb[:], cos[:])
                    nc.gpsimd.tensor_mul(gt[:], csb[:], ets)
                else:
                    ets = eth_sb[
                        :, ETH_BASE[p] + ETH_OFF[p][q] : ETH_BASE[p] + ETH_OFF[p][q] + 512
                    ]
                    csb = csp.tile([128, 512], BF16, tag="csb", name=f"csb{p}_{q}")
                    nc.scalar.copy(csb[:], cos[:])
                    defer.append((gt, csb, ets))
                pend.append((q, gt))
                while len(pend) > LAG:
                    pq, pgt = pend.pop(0)
                    emit_agg(pq, pgt, stop=False)
            for dgt, dcsb, dets in defer:
                nc.vector.tensor_mul(dgt[:], dcsb[:], dets)
            defer = []
            for k, (pq, pgt) in enumerate(pend):
                emit_agg(pq, pgt, stop=(k == len(pend) - 1))
            pend = []

            # relu epilogue: one FD=512 op + one out-DMA per pass (ScalarE is
            # the busiest engine, so no half-splitting)
            osl = slice(p * 512, (p + 1) * 512)
            nc.scalar.activation(out_sb[:, osl], outT[:], AF.Relu)
            nc.sync.dma_start(out[:, osl], out_sb[:, osl])

    nc.compile()
    return nc


_NC_CACHE = None


def _get_nc():
    global _NC_CACHE
    if _NC_CACHE is None:
        _NC_CACHE = build_nc()
    return _NC_CACHE


def make_in_maps(x, edge_weight, W, b):
    x = np.asarray(x, dtype=np.float32)
    edge_weight = np.asarray(edge_weight, dtype=np.float32)
    W = np.asarray(W, dtype=np.float32)
    b = np.asarray(b, dtype=np.float32)
    in_maps = []
    for core in CORE_IDS:
        h = x[core] @ W.T + b  # [N, OUT] fp32
        nrm = np.sqrt((h * h).sum(axis=-1, keepdims=True))
        hn = h / np.maximum(nrm, EPS)
        hnt = np.ascontiguousarray(hn.T)  # [IN=128, N]
        hp = np.ascontiguousarray(
            (h / 255.0).reshape(NQ, 128, OUT).transpose(1, 0, 2).reshape(128, NQ * OUT)
        )
        hh = np.concatenate([hnt, hp], axis=1).astype(ml_dtypes.bfloat16)
        ewt = edge_weight[core].T  # [src, dst]; block[i,c] = ew[p*512+c, q*128+i]
        ew255 = ewt * 255.0
        qblocks, hblocks = [], []
        for p in range(NP):
            for q in range(NQ):
                blk = ew255[q * 128 : (q + 1) * 128, p * 512 : (p + 1) * 512]
                if PATS[p][q] == "sv":
                    hblocks.append(blk.astype(ml_dtypes.bfloat16))
                else:
                    qblocks.append(np.round(blk).astype(np.uint8))
        etq = np.ascontiguousarray(np.concatenate(qblocks, axis=1))
        eth = np.ascontiguousarray(np.concatenate(hblocks, axis=1))
        in_maps.append({"hh": hh, "etq": etq, "eth": eth})
    return in_maps


def kernel(x, edge_weight, W, b):
    nc = _get_nc()
    in_maps = make_in_maps(x, edge_weight, W, b)
    res = run_bass_kernel_spmd(nc, in_maps, core_ids=CORE_IDS)
    out = np.stack(
        [
            np.ascontiguousarray(res.results[i]["out"].T).astype(np.float32)
            for i in range(len(CORE_IDS))
        ]
    )
    return out


# revision 17
# speedup vs baseline: 1.0735x; 1.0735x over previous
"""Trainium2 Bass kernel for AdaptConv-style GNN message passing.

Reference computation (per batch element b):
    h   = x @ W.T + b                       # [N, OUT]
    hn  = h / max(||h||_row, 1e-12)         # row-wise L2 normalize
    cos = hn @ hn.T                         # [N, N]
    out = relu((edge_weight * cos) @ h)     # [N, OUT]

Sharding: pure data-parallel over batch B=8 across the 8 NeuronCores
(no collectives).

Host-side preprocessing (linear+normalize is 0.8% of FLOPs, folded into
the input layout pass):
    hh = [hn.T | (h/255)-rowmajor-per-band]  bf16 [128, 2*N]
    edge weights are stored pre-transposed and pre-scaled by 255, split
    by gate class:
      etq uint8 [128, ...]: round(255*ew) blocks for DVE-direct and
          GpSimd gate bands (8-bit fixed point, sigma ~0.2%)
      eth bf16  [128, ...]: 255*ew blocks for the ScalarE-copy+DVE bands
          (bf16 keeps the DVE mul in 2x packed mode)
    The 1/255 de-scale rides for free in the hp half of hh.

On-chip dataflow per core: 4 column passes x 16 q-bands, FD=512
matmuls (bf16, fp32 PSUM):
    cos[q', 512p] = hnT[:,q]^T @ hnT[:, pass-cols]   (1 MM -> 1-bank PSUM)
    gt = et[q,p] * cos    per-band gate rotated over DVE-direct /
                          ScalarE-copy+DVE(deferred) / ScalarE-copy+GpSimd
    outT_p[:, :] += hp[q]^T @ gt                      (1 MM, PSUM accum)
    relu epilogue per pass in 2 halves (ScalarE, bf16) + DMA out.
outT is double-buffered (2x1 PSUM bank) so pass p+1 aggs never wait on
pass p's relu; cos tiles rotate over 6 single-bank PSUM buffers.

Warmup: ~32 scratch FD=128 matmuls open the HAM clock window (PE at
2.4GHz when real MMs start) and cover the hnT DMA flight time.

DMA: all on the Sync HWDGE ring, issued in first-use order.
"""

import ml_dtypes
import numpy as np

import concourse.mybir as mybir
import concourse.tile as tile
from concourse import bacc
from concourse.bass_utils import run_bass_kernel_spmd

B, N, IN, OUT = 8, 2048, 128, 128
NQ = N // 128          # 16 q-bands
NP = N // 512          # 4 column passes
FP32 = mybir.dt.float32
BF16 = mybir.dt.bfloat16
U8 = mybir.dt.uint8
AF = mybir.ActivationFunctionType
EPS = 1e-12

CORE_IDS = list(range(8))

N_WARMUP = 30  # ~3.2us of cold FD=128 matmuls: covers the HAM window and
               # keeps the PE busy until the hnT DMA lands
LAG = 8        # agg matmuls trail gates by this many bands (absorbs DVE/
               # ScalarE gate-queue jitter at ~95% engine occupancy)

# Gate engine class per (pass, band): v = DVE fp32-direct (u8 et),
# sv = ScalarE copy + deferred DVE bf16 mul (bf16 et), sg = ScalarE copy +
# GpSimd mul (u8 et).  Tails are v/sv so relu never waits on GpSimd; sg
# spaced >=2 so consecutive GpSimd muls don't queue.  Pass 0 pushes all
# sv bands late so the bf16 eth DMA can ride behind the u8/hh chunks
# during the slow DMA ramp.  Last pass drops a late sg (GpSimd latency
# unhideable at the kernel tail).
PATS = [
    ["v", "v", "sg", "v", "sg", "v", "sg", "v",
     "sg", "sv", "sv", "sg", "sv", "sv", "v", "v"],
    ["v", "v", "sg", "sv", "v", "sg", "sv", "v",
     "sg", "sv", "v", "sg", "sv", "sg", "v", "v"],
    ["v", "v", "sg", "sv", "v", "sg", "sv", "v",
     "sg", "sv", "v", "sg", "sv", "sg", "v", "v"],
    ["v", "v", "sg", "sv", "v", "sg", "sv", "v",
     "sg", "sv", "v", "sg", "sv", "sv", "v", "v"],
]

# per-pass column offsets of each band's et block inside etq (u8) / eth
# (bf16), in band order; blocks are stored in consumption order.
ETQ_OFF = []  # [pass][band] -> col offset or None
ETH_OFF = []
ETQ_PASS = []  # cols per pass in etq
ETH_PASS = []
for _p in range(NP):
    qo, ho = [], []
    qc = hc = 0
    for _q in range(NQ):
        if PATS[_p][_q] == "sv":
            ho.append(hc)
            qo.append(None)
            hc += 512
        else:
            qo.append(qc)
            ho.append(None)
            qc += 512
    ETQ_OFF.append(qo)
    ETH_OFF.append(ho)
    ETQ_PASS.append(qc)
    ETH_PASS.append(hc)
ETQ_BASE = [sum(ETQ_PASS[:p]) for p in range(NP + 1)]
ETH_BASE = [sum(ETH_PASS[:p]) for p in range(NP + 1)]


def build_nc():
    from contextlib import ExitStack

    nc = bacc.Bacc("TRN2", target_bir_lowering=False, debug=False, num_devices=8)

    hh = nc.dram_tensor("hh", [128, 2 * N], BF16, kind="ExternalInput").ap()
    etq = nc.dram_tensor("etq", [128, ETQ_BASE[NP]], U8, kind="ExternalInput").ap()
    eth = nc.dram_tensor("eth", [128, ETH_BASE[NP]], BF16, kind="ExternalInput").ap()
    out = nc.dram_tensor("out", [OUT, N], BF16, kind="ExternalOutput").ap()

    with tile.TileContext(nc) as tc, ExitStack() as ctx:
        singles = ctx.enter_context(tc.tile_pool(name="singles", bufs=1))
        gtp = ctx.enter_context(tc.tile_pool(name="gtp", bufs=10))
        csp = ctx.enter_context(tc.tile_pool(name="csp", bufs=6))
        cps_pool = ctx.enter_context(tc.tile_pool(name="cps", bufs=6, space="PSUM"))
        outp = ctx.enter_context(tc.tile_pool(name="outp", bufs=2, space="PSUM"))

        hh_sb = singles.tile([128, 2 * N], BF16, tag="hh_sb")
        etq_sb = singles.tile([128, ETQ_BASE[NP]], U8, tag="etq_sb")
        eth_sb = singles.tile([128, ETH_BASE[NP]], BF16, tag="eth_sb")
        out_sb = singles.tile([OUT, N], BF16, tag="out_sb")
        wsc = singles.tile([128, 128], BF16, tag="wsc")

        # warmup scratch memset on DVE (ready earliest) so warmup matmuls
        # start right after engine init; the tiny gpsimd tensor_mul forces
        # GpSimd's tensor-op library load now instead of in front of the
        # first real gate (kept off wsc so the PE does not wait on GpSimd).
        nc.vector.memset(wsc[:], 0.0)
        lib = singles.tile([1, 2], BF16, tag="lib")
        nc.vector.memset(lib[:], 0.0)
        nc.gpsimd.tensor_mul(lib[0:1, 0:2], lib[0:1, 0:2], lib[0:1, 0:2])

        # ---- DMAs, all on the Sync HWDGE ring, ordered by each chunk's
        # first-need time in the band schedule.  (Both two-ring splits were
        # tried and regressed: the rings share the 8 HWDGE in-flight sems
        # and the Scalar ring's issue ops block ScalarE's gate copies.) ----
        def etq_dma(p, c0, c1):
            csl = slice(ETQ_BASE[p] + c0, ETQ_BASE[p] + c1)
            nc.sync.dma_start(etq_sb[:, csl], etq[:, csl])

        def eth_dma(p):
            csl = slice(ETH_BASE[p], ETH_BASE[p + 1])
            nc.sync.dma_start(eth_sb[:, csl], eth[:, csl])

        def hh_dma(c0, c1):
            nc.sync.dma_start(hh_sb[:, c0:c1], hh[:, c0:c1])

        hh_dma(0, 1024)               # pass-0/1 rhs + lhsT bands 0-7
        etq_dma(0, 0, 512 * 9)        # pass-0 u8 blocks, bands 0-8
        hh_dma(2048, 2560)            # hp bands 0-3 (aggs start LAG bands in)
        hh_dma(1024, 2048)            # lhsT bands 8-15 + pass 2/3 rhs
        eth_dma(0)                    # pass-0 bf16 (sv@9,10,12,13) blocks
        hh_dma(2560, 3072)            # hp bands 4-7
        etq_dma(0, 512 * 9, ETQ_PASS[0])  # pass-0 u8 blocks, bands 11,14,15
        hh_dma(3072, 4096)            # hp bands 8-15
        etq_dma(1, 0, 512 * 9)        # pass-1 u8 blocks, bands 0-11
        eth_dma(1)
        etq_dma(1, 512 * 9, ETQ_PASS[1])
        etq_dma(2, 0, ETQ_PASS[2])
        eth_dma(2)
        etq_dma(3, 0, ETQ_PASS[3])
        eth_dma(3)

        # ---- HAM warmup ----
        outT0 = outp.tile([OUT, 512], FP32, tag="outT", name="outT0")
        for _ in range(N_WARMUP):
            nc.tensor.matmul(
                outT0[:, 0:128], wsc[:], wsc[:],
                start=True, stop=True, skip_group_check=True,
            )

        # ---- main loop ----
        for p in range(NP):
            outT = outT0 if p == 0 else outp.tile(
                [OUT, 512], FP32, tag="outT", name=f"outT{p}"
            )
            PAT = PATS[p]
            rhs = hh_sb[:, p * 512 : (p + 1) * 512]
            pend = []
            defer = []

            def emit_agg(q, gt, stop):
                hpq = hh_sb[:, N + q * 128 : N + (q + 1) * 128]
                nc.tensor.matmul(
                    outT[:], hpq, gt[:],
                    start=(q == 0), stop=stop, skip_group_check=True,
                )

            for q in range(NQ):
                hnq = hh_sb[:, q * 128 : (q + 1) * 128]
                cos = cps_pool.tile([128, 512], FP32, tag="cos", name=f"cos{p}_{q}")
                nc.tensor.matmul(cos[:], hnq, rhs, start=True, stop=True)
                for dgt, dcsb, dets in defer:
                    nc.vector.tensor_mul(dgt[:], dcsb[:], dets)
                defer = []
                gt = gtp.tile([128, 512], BF16, tag="gt", name=f"gt{p}_{q}")
                cls = PAT[q]
                if cls == "v":
                    ets = etq_sb[
                        :, ETQ_BASE[p] + ETQ_OFF[p][q] : ETQ_BASE[p] + ETQ_OFF[p][q] + 512
                    ]
                    nc.vector.tensor_mul(gt[:], cos[:], ets)
                elif cls == "sg":
                    ets = etq_sb[
                        :, ETQ_BASE[p] + ETQ_OFF[p][q] : ETQ_BASE[p] + ETQ_OFF[p][q] + 512
                    ]
                    csb = csp.tile([128, 512], BF16, tag="csb", name=f"csb{p}_{q}")
                    nc.scalar.copy(csb[:], cos[:])
                    nc.gpsimd.tensor_mul(gt[:], csb[:], ets)
                else:
                    ets = eth_sb[
                        :, ETH_BASE[p] + ETH_OFF[p][q] : ETH_BASE[p] + ETH_OFF[p][q] + 512
                    ]
                    csb = csp.tile([128, 512], BF16, tag="csb", name=f"csb{p}_{q}")
                    nc.scalar.copy(csb[:], cos[:])
                    defer.append((gt, csb, ets))
                pend.append((q, gt))
                while len(pend) > LAG:
                    pq, pgt = pend.pop(0)
                    emit_agg(pq, pgt, stop=False)
            for dgt, dcsb, dets in defer:
                nc.vector.tensor_mul(dgt[:], dcsb[:], dets)
            defer = []
            for k, (pq, pgt) in enumerate(pend):
                emit_agg(pq, pgt, stop=(k == len(pend) - 1))
            pend = []

            # relu epilogue: one FD=512 op + one out-DMA per pass (ScalarE is
            # the busiest engine, so no half-splitting)
            osl = slice(p * 512, (p + 1) * 512)
            nc.scalar.activation(out_sb[:, osl], outT[:], AF.Relu)
            nc.sync.dma_start(out[:, osl], out_sb[:, osl])

    nc.compile()
    return nc


_NC_CACHE = None


def _get_nc():
    global _NC_CACHE
    if _NC_CACHE is None:
        _NC_CACHE = build_nc()
    return _NC_CACHE


def make_in_maps(x, edge_weight, W, b):
    x = np.asarray(x, dtype=np.float32)
    edge_weight = np.asarray(edge_weight, dtype=np.float32)
    W = np.asarray(W, dtype=np.float32)
    b = np.asarray(b, dtype=np.float32)
    in_maps = []
    for core in CORE_IDS:
        h = x[core] @ W.T + b  # [N, OUT] fp32
        nrm = np.sqrt((h * h).sum(axis=-1, keepdims=True))
        hn = h / np.maximum(nrm, EPS)
        hnt = np.ascontiguousarray(hn.T)  # [IN=128, N]
        hp = np.ascontiguousarray(
            (h / 255.0).reshape(NQ, 128, OUT).transpose(1, 0, 2).reshape(128, NQ * OUT)
        )
        hh = np.concatenate([hnt, hp], axis=1).astype(ml_dtypes.bfloat16)
        ewt = edge_weight[core].T  # [src, dst]; block[i,c] = ew[p*512+c, q*128+i]
        ew255 = ewt * 255.0
        qblocks, hblocks = [], []
        for p in range(NP):
            for q in range(NQ):
                blk = ew255[q * 128 : (q + 1) * 128, p * 512 : (p + 1) * 512]
                if PATS[p][q] == "sv":
                    hblocks.append(blk.astype(ml_dtypes.bfloat16))
                else:
                    qblocks.append(np.round(blk).astype(np.uint8))
        etq = np.ascontiguousarray(np.concatenate(qblocks, axis=1))
        eth = np.ascontiguousarray(np.concatenate(hblocks, axis=1))
        in_maps.append({"hh": hh, "etq": etq, "eth": eth})
    return in_maps


def kernel(x, edge_weight, W, b):
    nc = _get_nc()
    in_maps = make_in_maps(x, edge_weight, W, b)
    res = run_bass_kernel_spmd(nc, in_maps, core_ids=CORE_IDS)
    out = np.stack(
        [
            np.ascontiguousarray(res.results[i]["out"].T).astype(np.float32)
            for i in range(len(CORE_IDS))
        ]
    )
    return out
